# revision 54
# baseline (speedup 1.0000x reference)
"""2-layer GAT (PyG semantics) on 8 Trainium2 NeuronCores via Bass/Tile.

Strategy:
  - dst-sharded edges: core k owns dst in [k*12500, (k+1)*12500)
  - pass0: h1|s1|d1 = x @ [W1|Wsrc|Wdst] node-sharded; rows packed as
    768B fp16 table rows [ (h+b|1)x4heads | s_f32 | d_f16 ]; AllGather
  - edge pass: dma_gather (int16, quarter-sliced table) streams 128-edge
    tiles; attention p = exp(lrelu(s_src + d_dst)) with d expanded via
    one-hot matmul; segment-sum via one-hot matmul into per-128-dst-block
    PSUM; denominators via baked ones-columns; biases pre-folded.
  - layer-2 dense (W2cat) fused into layer-1 block epilogue; second
    AllGather; same machinery with H=1; classifier + log_softmax fused.
"""
import sys
sys.path.insert(0, "/opt/trn_rl_repo")
import numpy as np

import concourse.bass as bass
import concourse.bacc as bacc
import concourse.tile as tile
from concourse import mybir, library_config
from concourse import bass_utils

P = 128
NEG_SLOPE = 0.2
N_NODES = 100000
FIN = 165
HID = 64
H1 = 4
NC_OUT = 2
NCORES = 8
NPC = N_NODES // NCORES          # 12500
NBLK = (NPC + P - 1) // P        # 98
QROWS = 25000
NQ = 4
CH = NPC // NQ                   # 3125: local rows per table chunk
RU1 = 384                        # fp16 units per L1 row (768B)
RU2 = 128                        # fp16 units per L2 row (256B)
S1F = 130                        # f32 col of s in L1 row (unit 260)
S2F = 33                         # f32 col of s2 in L2 row (unit 66)
GM = 8                           # tiles per e/p/M batch

f16 = mybir.dt.float16
f32 = mybir.dt.float32
f8 = mybir.dt.float8e4
i16 = mybir.dt.int16
AF = mybir.ActivationFunctionType
ALU = mybir.AluOpType
AXX = mybir.AxisListType.X


def _host_prep(edge_index):
    src = np.concatenate([np.asarray(edge_index[0]), np.arange(N_NODES)]).astype(np.int64)
    dst = np.concatenate([np.asarray(edge_index[1]), np.arange(N_NODES)]).astype(np.int64)
    core_of = dst // NPC
    percore = []
    cnt = np.zeros((NCORES, NBLK, NQ), np.int64)
    for k in range(NCORES):
        sel = np.nonzero(core_of == k)[0]
        s_k = src[sel]
        d_k = dst[sel] - k * NPC
        blk = d_k >> 7
        # chunk-major table: node n sits at chunk c=(n%NPC)//CH, local idx
        # (n//NPC)*CH + (n%NPC)%CH — lets AllGather chunk c start as soon as
        # local rows [c*CH,(c+1)*CH) exist
        q = (s_k % NPC) // CH
        # within (block, chunk): ascending table address for DRAM locality
        perm_idx = (s_k // NPC) * CH + (s_k % NPC) % CH
        order = np.lexsort((perm_idx, q, blk))
        s_k, d_k, blk, q = s_k[order], d_k[order], blk[order], q[order]
        percore.append((s_k, d_k, blk, q))
        np.add.at(cnt[k], (blk, q), 1)
    reg = cnt.max(axis=0)                       # shared padded count per (b,q)
    tiles_bq = (reg + P - 1) // P               # may be 0
    ntiles = int(tiles_bq.sum())
    calls = [(b, qq, int(tiles_bq[b, qq]))
             for b in range(NBLK) for qq in range(NQ) if tiles_bq[b, qq] > 0]
    plan = {"reg": reg, "tiles_bq": tiles_bq, "ntiles": ntiles,
            "ncalls": len(calls)}

    data = []
    for k in range(NCORES):
        s_k, d_k, blk, q = percore[k]
        srcs = np.zeros(ntiles * P, np.int64)
        dloc = np.full(ntiles * P, -1, np.int64)
        gcnt = np.zeros(len(calls), np.int32)
        t0 = 0
        ci = 0
        # per-(b,q): real edges then a -1 (skipped) pad tail
        ptr = {}
        for b in range(NBLK):
            for qq in range(NQ):
                T = int(tiles_bq[b, qq])
                if T == 0:
                    continue
                m = (blk == b) & (q == qq)
                cs = s_k[m]; cd = d_k[m]
                L = len(cs)
                base = t0 * P
                # chunk-local permuted index (0..QROWS), -1 for skipped pads
                srcs[base:base + L] = (cs // NPC) * CH + (cs % NPC) % CH
                dloc[base:base + L] = cd & 127
                srcs[base + L:base + T * P] = -1
                gcnt[ci] = L
                ci += 1
                ptr[(b, qq)] = (t0, T)
                t0 += T
        assert t0 == ntiles and ci == len(calls)
        srcs_t = srcs.reshape(ntiles, P)
        dloc_t = dloc.reshape(ntiles, P)

        # D^T [edge j, row i] and D [row i, edge j] per tile, packed [P, 256];
        # stored p-major [P, ntiles, 256] so per-block DMA reads are one long
        # contiguous span per partition (no sub-512B descriptor penalty)
        import ml_dtypes
        DD = np.zeros((ntiles, P, 2 * P), ml_dtypes.float8_e4m3)
        for t in range(ntiles):
            dl = dloc_t[t]
            v = np.nonzero(dl >= 0)[0]
            DD[t, v, dl[v]] = 1.0            # cols 0:128  = D^T
            DD[t, dl[v], P + v] = 1.0        # cols 128:256 = D
        DD = np.ascontiguousarray(DD.transpose(1, 0, 2))
        # int16 quarter-local idx, wrapped [16, n/16] then replicated to 128
        idx16 = np.zeros((16, ntiles * P // 16), np.int16)
        for b in range(NBLK):
            for qq in range(NQ):
                if (b, qq) not in ptr:
                    continue
                t0_, T = ptr[(b, qq)]
                n = T * P
                flat = srcs_t.reshape(-1)[t0_ * P: t0_ * P + n]
                assert (flat >= -1).all() and (flat < QROWS).all()
                idx16[:, t0_ * P // 16:(t0_ * P + n) // 16] = \
                    flat.reshape(n // 16, 16).T.astype(np.int16)
        data.append(dict(idx_stream=np.tile(idx16, (8, 1)), DD=DD,
                         gcnt=gcnt.reshape(1, -1)))
    return plan, data


def _host_weights(W1, a_src1, a_dst1, W2, a_src2, a_dst2):
    # layer-1 h features stored head-minor: row unit c*H1+h for c<64,
    # ones at 256:260, s-proj at 260+h, d-proj at 264+h
    W1 = np.asarray(W1, np.float32); W2 = np.asarray(W2, np.float32)
    a_src1 = np.asarray(a_src1, np.float32); a_dst1 = np.asarray(a_dst1, np.float32)
    a_src2 = np.asarray(a_src2, np.float32); a_dst2 = np.asarray(a_dst2, np.float32)
    Wcat1 = np.zeros((FIN, 268), np.float32)
    for h in range(H1):
        Wcat1[:, h:256:H1] = W1[:, h * HID:(h + 1) * HID]
        Wcat1[:, 260 + h] = W1[:, h * HID:(h + 1) * HID] @ a_src1[h]
        Wcat1[:, 264 + h] = W1[:, h * HID:(h + 1) * HID] @ a_dst1[h]
    Wcat2 = np.zeros((H1 * HID, 66), np.float32)
    # W2 rows permuted to the (c,h) feature order
    perm = np.arange(H1 * HID).reshape(H1, HID).T.reshape(-1)  # [c*H1+h] -> h*HID+c
    Wcat2[:, :HID] = W2[perm]
    Wcat2[:, 64] = W2[perm] @ a_src2[0]
    Wcat2[:, 65] = W2[perm] @ a_dst2[0]
    return Wcat1.astype(np.float16), Wcat2.astype(np.float16)


def _build(plan, sim=False):
    reg = plan["reg"]; tiles_bq = plan["tiles_bq"]; ntiles = plan["ntiles"]

    nc = bacc.Bacc("TRN2", target_bir_lowering=False, debug=False,
                   enable_asserts=False, num_devices=1 if sim else NCORES,
                   num_swdge_queues=4)

    x_in = nc.dram_tensor("xT", [FIN, NPC], f16, kind="ExternalInput")
    w1_in = nc.dram_tensor("w1cat", [FIN, 268], f16, kind="ExternalInput")
    w2_in = nc.dram_tensor("w2cat", [H1 * HID, 66], f16, kind="ExternalInput")
    b1_in = nc.dram_tensor("b1", [1, H1 * HID], f32, kind="ExternalInput")
    b2_in = nc.dram_tensor("b2", [1, HID], f32, kind="ExternalInput")
    wc_in = nc.dram_tensor("wc", [1, HID * NC_OUT], f32, kind="ExternalInput")
    bc_in = nc.dram_tensor("bc", [1, NC_OUT], f32, kind="ExternalInput")
    idx_in = nc.dram_tensor("idxs", [P, ntiles * P // 16], i16, kind="ExternalInput")
    dd_in = nc.dram_tensor("dds", [P, ntiles, 2 * P], f8, kind="ExternalInput")
    gcnt_in = nc.dram_tensor("gcnt", [1, plan["ncalls"]], mybir.dt.int32,
                             kind="ExternalInput")
    out_t = nc.dram_tensor("out", [NPC, NC_OUT], f32, kind="ExternalOutput")

    RG = [list(range(NCORES))]

    with tile.TileContext(nc) as tc:
        with tc.tile_pool(name="const", bufs=1) as cp, \
             tc.tile_pool(name="work", bufs=3) as wp, \
             tc.tile_pool(name="gst", bufs=3) as gp, \
             tc.tile_pool(name="dts", bufs=8) as dp, \
             tc.tile_pool(name="dram", bufs=1, space="DRAM") as dr, \
             tc.tile_pool(name="psA", bufs=3, space="PSUM") as psA, \
             tc.tile_pool(name="psB", bufs=2, space="PSUM") as psB, \
             tc.tile_pool(name="psC", bufs=2, space="PSUM") as psC, \
             tc.tile_pool(name="psD", bufs=1, space="PSUM") as psD:

            nc.gpsimd.load_library(library_config.mlp)

            hs1_loc = dr.tile([NPC, RU1], f16)
            hs1_full = dr.tile([N_NODES, RU1], f16)
            hs2_loc = dr.tile([NPC, RU2], f16)
            hs2_full = dr.tile([N_NODES, RU2], f16)

            # ---------- constants
            from concourse.masks import make_identity
            ident = cp.tile([P, P], f32)
            make_identity(nc, ident[:])
            w1c = cp.tile([P, 268], f16)
            w1c2 = cp.tile([P, 268], f16)
            nc.sync.dma_start(out=w1c[:], in_=w1_in[0:128, :])
            nc.sync.dma_start(out=w1c2[0:FIN - 128, :], in_=w1_in[128:FIN, :])
            w2c = cp.tile([P, 66], f16)
            w2c2 = cp.tile([P, 66], f16)
            nc.sync.dma_start(out=w2c[:], in_=w2_in[0:128, :])
            nc.sync.dma_start(out=w2c2[:], in_=w2_in[128:256, :])
            onecol = cp.tile([1, P], f16)
            nc.vector.memset(onecol[:], 1.0)
            gcnt_t = cp.tile([1, plan["ncalls"]], mybir.dt.int32)
            nc.sync.dma_start(out=gcnt_t[:], in_=gcnt_in[:])
            gregs = [nc.alloc_register(mybir.EngineType.Pool, f"gcnt_r{i}")
                     for i in range(4)]

            def replicate(dram_ap, ncols, tag):
                srcf = wp.tile([1, 256], f16, tag="repf16")
                srci = wp.tile([1, 256], f32, tag="repf32")
                nc.sync.dma_start(out=srci[:, :ncols], in_=dram_ap)
                nc.vector.tensor_copy(out=srcf[:, :ncols], in_=srci[:, :ncols])
                ps = psD.tile([P, 512], f32, tag="scr")
                nc.tensor.matmul(out=ps[:, :ncols], lhsT=onecol[:], rhs=srcf[:, :ncols],
                                 start=True, stop=True)
                dst = cp.tile([P, ncols], f32, tag=tag)
                nc.vector.tensor_copy(out=dst[:], in_=ps[:, :ncols])
                return dst

            b1rep = replicate(b1_in[:], H1 * HID, "b1rep")
            b2rep = replicate(b2_in[:], HID, "b2rep")
            wcrep = replicate(wc_in[:], HID * NC_OUT, "wcrep")
            bcrep = replicate(bc_in[:], NC_OUT, "bcrep")

            logits = cp.tile([P, NBLK * NC_OUT], f32)

            # ================= pass 0 =================
            for nt in range(NBLK):
                r0 = nt * P
                rows = min(P, NPC - r0)
                xT1 = wp.tile([P, P], f16, tag="xT1")
                xT2 = wp.tile([P, P], f16, tag="xT2")
                if rows < P:
                    nc.vector.memset(xT1[:], 0.0)
                    nc.vector.memset(xT2[:], 0.0)
                nc.sync.dma_start(out=xT1[:, 0:rows], in_=x_in[0:P, r0:r0 + rows])
                nc.sync.dma_start(out=xT2[0:FIN - P, 0:rows],
                                  in_=x_in[P:FIN, r0:r0 + rows])
                acc = psA.tile([P, 268], f32, tag="agg1")
                nc.tensor.matmul(out=acc[:], lhsT=xT1[:], rhs=w1c[:], start=True, stop=False)
                nc.tensor.matmul(out=acc[:], lhsT=xT2[0:FIN - P, :], rhs=w1c2[0:FIN - P, :],
                                 start=False, stop=True)
                row = wp.tile([P, RU1], f16, tag="row1")
                nc.vector.tensor_tensor(
                    out=row[:, 0:256], in0=acc[:, 0:256], in1=b1rep[:], op=ALU.add)
                if nt < 3:
                    # row tile cycles 3 pool buffers; constants persist after
                    nc.vector.memset(row[:, 256:260], 1.0)
                    nc.vector.memset(row[:, 272:RU1], 0.0)
                rowf = row[:].bitcast(f32)
                nc.vector.tensor_copy(out=rowf[:, S1F:S1F + H1], in_=acc[:, 260:264])
                nc.vector.tensor_copy(out=row[:, 268:268 + H1], in_=acc[:, 264:268])
                nc.sync.dma_start(out=hs1_loc[r0:r0 + rows, :], in_=row[0:rows, :])

            for c in range(NQ):
                if sim:
                    nc.sync.dma_start(
                        out=hs1_full[c * QROWS:c * QROWS + CH, :],
                        in_=hs1_loc[c * CH:(c + 1) * CH, :])
                else:
                    nc.gpsimd.collective_compute(
                        "AllGather", ALU.bypass, replica_groups=RG,
                        ins=[hs1_loc[c * CH:(c + 1) * CH, :]],
                        outs=[hs1_full[c * QROWS:(c + 1) * QROWS, :]])

            # ================= shared epilogue helpers =================
            def _elu(dst, src_ap, ncols, tagp):
                mn = wp.tile([P, ncols], f32, tag=f"{tagp}mn")
                nc.vector.tensor_scalar(mn[:], src_ap, 0.0, None, ALU.min)
                ex = wp.tile([P, ncols], f32, tag=f"{tagp}ex")
                nc.scalar.activation(out=ex[:], in_=mn[:], func=AF.Exp)
                nc.vector.tensor_scalar(ex[:], ex[:], 1.0, None, ALU.subtract)
                nc.vector.tensor_tensor(out=dst, in0=src_ap, in1=ex[:], op=ALU.max)

            def _epilogue1(b, r0, rows, aggp):
                rec = wp.tile([P, H1], f32, tag="rec1")
                nc.vector.reciprocal(out=rec[:], in_=aggp[:, 256:260])
                nrm = wp.tile([P, H1 * HID], f32, tag="nrm1")
                nc.vector.tensor_tensor(
                    out=nrm[:].rearrange("p (c h) -> p c h", h=H1),
                    in0=aggp[:, 0:256].rearrange("p (c h) -> p c h", h=H1),
                    in1=rec[:].unsqueeze(1).to_broadcast([P, HID, H1]),
                    op=ALU.mult)
                h2in = wp.tile([P, H1 * HID], f32, tag="h2in")
                _elu(h2in[:], nrm[:], H1 * HID, "e1")
                pt1 = psD.tile([P, 512], f32, tag="scr")
                h2T1 = wp.tile([P, P], f16, tag="h2T1")
                nc.tensor.transpose(out=pt1[:, 0:P], in_=h2in[:, 0:P], identity=ident[:])
                nc.vector.tensor_copy(out=h2T1[:], in_=pt1[:, 0:P])
                pt2 = psD.tile([P, 512], f32, tag="scr")
                h2T2 = wp.tile([P, P], f16, tag="h2T2")
                nc.tensor.transpose(out=pt2[:, 0:P], in_=h2in[:, P:2 * P], identity=ident[:])
                nc.vector.tensor_copy(out=h2T2[:], in_=pt2[:, 0:P])
                mm2 = psB.tile([P, 66], f32, tag="agg2")
                nc.tensor.matmul(out=mm2[:], lhsT=h2T1[:], rhs=w2c[:], start=True, stop=False)
                nc.tensor.matmul(out=mm2[:], lhsT=h2T2[:], rhs=w2c2[:], start=False, stop=True)
                row2 = wp.tile([P, RU2], f16, tag="row2")
                nc.vector.tensor_tensor(out=row2[:, 0:HID], in0=mm2[:, 0:HID],
                                        in1=b2rep[:], op=ALU.add)
                if b < 3:
                    nc.vector.memset(row2[:, HID:HID + 2], 1.0)
                    nc.vector.memset(row2[:, 69:RU2], 0.0)
                row2f = row2[:].bitcast(f32)
                nc.vector.tensor_copy(out=row2f[:, S2F:S2F + 1], in_=mm2[:, 64:65])
                nc.vector.tensor_copy(out=row2[:, 68:69], in_=mm2[:, 65:66])
                nc.sync.dma_start(out=hs2_loc[r0:r0 + rows, :], in_=row2[0:rows, :])

            def _epilogue2(b, r0, rows, aggp):
                rec = wp.tile([P, 1], f32, tag="rec2")
                nc.vector.reciprocal(out=rec[:], in_=aggp[:, 64:65])
                nrm = wp.tile([P, HID], f32, tag="nrm2")
                nc.vector.tensor_tensor(
                    out=nrm[:], in0=aggp[:, 0:HID],
                    in1=rec[:].to_broadcast([P, HID]), op=ALU.mult)
                h3 = wp.tile([P, HID], f32, tag="h3")
                _elu(h3[:], nrm[:], HID, "e2")
                tmp = wp.tile([P, HID], f32, tag="lgt")
                wcv = wcrep[:].rearrange("p (k c) -> p k c", c=NC_OUT)
                lg = logits[:, b * NC_OUT:(b + 1) * NC_OUT]
                for c in range(NC_OUT):
                    nc.vector.tensor_tensor(out=tmp[:], in0=h3[:],
                                            in1=wcv[:, :, c], op=ALU.mult)
                    nc.vector.tensor_reduce(out=lg[:, c:c + 1], in_=tmp[:],
                                            op=ALU.add, axis=AXX)
                nc.vector.tensor_tensor(out=lg, in0=lg, in1=bcrep[:], op=ALU.add)

            def _final_logsoftmax():
                # batched log_softmax over logits [P, NBLK, NC_OUT]
                lgv = logits[:].rearrange("p (b c) -> p b c", c=NC_OUT)
                mx = wp.tile([P, NBLK], f32, tag="fmx")
                nc.vector.tensor_reduce(out=mx[:].unsqueeze(2), in_=lgv,
                                        op=ALU.max, axis=AXX)
                nc.vector.tensor_tensor(
                    out=lgv, in0=lgv,
                    in1=mx[:].unsqueeze(2).to_broadcast([P, NBLK, NC_OUT]),
                    op=ALU.subtract)
                exs = wp.tile([P, NBLK * NC_OUT], f32, tag="fexs")
                nc.scalar.activation(out=exs[:], in_=logits[:], func=AF.Exp)
                sm = wp.tile([P, NBLK], f32, tag="fsm")
                nc.vector.tensor_reduce(
                    out=sm[:].unsqueeze(2),
                    in_=exs[:].rearrange("p (b c) -> p b c", c=NC_OUT),
                    op=ALU.add, axis=AXX)
                lsm = wp.tile([P, NBLK], f32, tag="flsm")
                nc.scalar.activation(out=lsm[:], in_=sm[:], func=AF.Ln)
                nc.vector.tensor_tensor(
                    out=lgv, in0=lgv,
                    in1=lsm[:].unsqueeze(2).to_broadcast([P, NBLK, NC_OUT]),
                    op=ALU.subtract)

            # ================= edge loops =================
            maxTb = int(tiles_bq.sum(axis=1).max())

            def edge_layer(layer):
                RU = RU1 if layer == 1 else RU2
                NH = H1 if layer == 1 else 1
                MW = 260 if layer == 1 else 65
                SF = S1F if layer == 1 else S2F
                loc_tab = hs1_loc if layer == 1 else hs2_loc
                DOFF = 268 if layer == 1 else 68
                table = hs1_full if layer == 1 else hs2_full
                Tb_list = [int(tiles_bq[bb].sum()) for bb in range(NBLK)]
                t_glob = 0
                ci = 0
                for b in range(NBLK):
                    Tb = Tb_list[b]
                    G = gp.tile([P, maxTb, RU], f16, tag=f"G{layer}")
                    if b < 3:
                        # init all cycled buffers so skipped pad slots hold
                        # finite values (later blocks inherit real rows)
                        nc.vector.memset(G[:], 0.0)
                    idxs = dp.tile([P, (maxTb * P) // 16], i16, tag=f"ix{layer}")
                    nc.sync.dma_start(
                        out=idxs[:, 0:(Tb * P) // 16],
                        in_=idx_in[:, (t_glob * P) // 16:
                                   ((t_glob + Tb) * P) // 16])
                    ddb = gp.tile([P, maxTb, 2 * P], f8, tag=f"DDB{layer}")
                    nc.sync.dma_start(
                        out=ddb[:, 0:Tb, :],
                        in_=dd_in[:, t_glob:t_glob + Tb, :])
                    poff = 0
                    tt = 0
                    for qq in range(NQ):
                        T = int(tiles_bq[b, qq])
                        if T == 0:
                            continue
                        cv = gregs[ci % 4]
                        nc.gpsimd.reg_load(cv, gcnt_t[0:1, ci:ci + 1])
                        ci += 1
                        nc.gpsimd.dma_gather(
                            G[:, tt:tt + T, :],
                            table[qq * QROWS:(qq + 1) * QROWS, :],
                            idxs[:, ((poff + tt) * P) // 16:
                                 ((poff + tt + T) * P) // 16],
                            T * P, cv, RU,
                            queue_num=(b * NQ + qq) % 4,
                        )
                        tt += T
                    dblk = wp.tile([P, NH], f16, tag=f"dblk{layer}")
                    drows = min(P, NPC - b * P)
                    if drows < P:
                        nc.vector.memset(dblk[:], 0.0)
                    nc.sync.dma_start(
                        out=dblk[0:drows, :],
                        in_=loc_tab[b * P:b * P + drows, DOFF:DOFF + NH])
                    aggp = (psA if layer == 1 else psB).tile(
                        [P, 268 if layer == 1 else 66], f32,
                        tag="agg1" if layer == 1 else "agg2")

                    t = 0
                    while t < Tb:
                        gmt = min(GM, Tb - t)
                        dxp = psC.tile([P, GM * H1], f32, tag="dx")
                        ddg = ddb[:, poff:poff + Tb, :]
                        for ti in range(gmt):
                            nc.tensor.matmul(
                                out=dxp[:, ti * NH:(ti + 1) * NH],
                                lhsT=ddg[:, t + ti, P:2 * P], rhs=dblk[:],
                                start=True, stop=True, skip_group_check=True)
                        ee = wp.tile([P, GM * NH], f32, tag=f"e{layer}")
                        Gf = G[:].bitcast(f32)
                        nc.vector.tensor_tensor(
                            out=ee[:, :gmt * NH],
                            in0=Gf[:, t:t + gmt, SF:SF + NH],
                            in1=dxp[:, :gmt * NH], op=ALU.add)
                        lr = wp.tile([P, GM * NH], f32, tag=f"lr{layer}")
                        nc.scalar.activation(out=lr[:, :gmt * NH], in_=ee[:, :gmt * NH],
                                             func=AF.Prelu, alpha=NEG_SLOPE)
                        # f16 so the M multiply hits DVE 2x 16-bit mode
                        pp = wp.tile([P, GM * NH], f16, tag=f"pp{layer}")
                        nc.scalar.activation(out=pp[:, :gmt * NH], in_=lr[:, :gmt * NH],
                                             func=AF.Exp)
                        M = wp.tile([P, GM, MW], f16, tag=f"M{layer}")
                        if layer == 1:
                            # head-minor (c,h) layout: broadcast on middle dim
                            # keeps last dim packed -> DVE 2x/4x 16-bit mode
                            nc.vector.tensor_tensor(
                                out=M[:, 0:gmt, :].rearrange(
                                    "p t (c h) -> p t c h", h=NH),
                                in0=G[:, t:t + gmt, 0:MW].rearrange(
                                    "p t (c h) -> p t c h", h=NH),
                                in1=pp[:].rearrange("p (t h) -> p t h", h=NH)
                                    [:, 0:gmt, :].unsqueeze(2)
                                    .to_broadcast([P, gmt, 65, NH]),
                                op=ALU.mult)
                        else:
                            nc.vector.tensor_tensor(
                                out=M[:, 0:gmt, :],
                                in0=G[:, t:t + gmt, 0:MW],
                                in1=pp[:, 0:gmt].unsqueeze(2)
                                    .to_broadcast([P, gmt, MW]),
                                op=ALU.mult)
                        for ti in range(gmt):
                            nc.tensor.matmul(
                                out=aggp[:, 0:MW], lhsT=ddg[:, t + ti, 0:P],
                                rhs=M[:, ti, :],
                                start=(t + ti == 0), stop=(t + ti == Tb - 1),
                                skip_group_check=True)
                        t += gmt

                    r0 = b * P
                    rows = min(P, NPC - r0)
                    if layer == 1:
                        _epilogue1(b, r0, rows, aggp)
                    else:
                        _epilogue2(b, r0, rows, aggp)
                    t_glob += Tb

            edge_layer(1)
            for c in range(NQ):
                if sim:
                    nc.sync.dma_start(
                        out=hs2_full[c * QROWS:c * QROWS + CH, :],
                        in_=hs2_loc[c * CH:(c + 1) * CH, :])
                else:
                    nc.gpsimd.collective_compute(
                        "AllGather", ALU.bypass, replica_groups=RG,
                        ins=[hs2_loc[c * CH:(c + 1) * CH, :]],
                        outs=[hs2_full[c * QROWS:(c + 1) * QROWS, :]])
            edge_layer(2)
            _final_logsoftmax()

            nc.sync.dma_start(
                out=bass.AP(out_t[:].tensor, 0,
                            [[NC_OUT, P], [P * NC_OUT, NBLK - 1], [1, NC_OUT]]),
                in_=logits[:, 0:(NBLK - 1) * NC_OUT])
            lastrows = NPC - (NBLK - 1) * P
            nc.sync.dma_start(
                out=bass.AP(out_t[:].tensor, (NBLK - 1) * P * NC_OUT,
                            [[NC_OUT, lastrows], [1, NC_OUT]]),
                in_=logits[0:lastrows, (NBLK - 1) * NC_OUT:NBLK * NC_OUT])

    nc.compile()
    return nc


_CACHE = {}


def kernel(**inputs):
    x = np.asarray(inputs["x"], np.float32)
    edge_index = np.asarray(inputs["edge_index"])
    ekey = (edge_index.shape, int(edge_index[:, ::997].astype(np.int64).sum()))
    if _CACHE.get("ekey") != ekey:
        plan, data = _host_prep(edge_index)
        _CACHE.update(ekey=ekey, plan=plan, data=data, nc=None)
    plan, data = _CACHE["plan"], _CACHE["data"]
    Wcat1, Wcat2 = _host_weights(
        inputs["W1"], inputs["a_src1"], inputs["a_dst1"],
        inputs["W2"], inputs["a_src2"], inputs["a_dst2"])
    perm = np.arange(H1 * HID).reshape(H1, HID).T.reshape(-1)
    b1 = np.asarray(inputs["b1"], np.float32).reshape(-1)[perm].reshape(1, -1)
    b2 = np.asarray(inputs["b2"], np.float32).reshape(1, -1)
    wc = np.asarray(inputs["Wc"], np.float32).reshape(1, -1).copy()
    bc = np.asarray(inputs["bc"], np.float32).reshape(1, -1)

    if _CACHE.get("nc") is None:
        _CACHE["nc"] = _build(plan)
    nc = _CACHE["nc"]
    in_maps = []
    for k in range(NCORES):
        in_maps.append({
            "xT": np.ascontiguousarray(
                x[k * NPC:(k + 1) * NPC].T).astype(np.float16),
            "w1cat": Wcat1, "w2cat": Wcat2,
            "b1": b1, "b2": b2, "wc": wc, "bc": bc,
            "idxs": data[k]["idx_stream"],
            "dds": data[k]["DD"],
            "gcnt": data[k]["gcnt"],
        })
    res = bass_utils.run_bass_kernel_spmd(
        nc, in_maps, core_ids=list(range(NCORES)),
        trace=globals().get("TRACE", False))
    globals()["LAST_RES"] = res
    globals()["LAST_NC"] = nc
    globals()["LAST_IN_MAPS"] = in_maps
    out = np.concatenate([np.asarray(r["out"], np.float32) for r in res.results], axis=0)
    return out


if __name__ == "__main__":
    # smoke test with tiny synthetic graph shape is not supported (shapes
    # hardcoded); run via test.py on the real problem.
    pass



# revision 58
# speedup vs baseline: 1.0945x; 1.0945x over previous
"""2-layer GAT (PyG semantics) on 8 Trainium2 NeuronCores via Bass/Tile.

Strategy:
  - dst-sharded edges: core k owns dst in [k*12500, (k+1)*12500)
  - pass0: h1|s1|d1 = x @ [W1|Wsrc|Wdst] node-sharded; rows packed as
    768B fp16 table rows [ (h+b|1)x4heads | s_f32 | d_f16 ]; AllGather
  - edge pass: dma_gather (int16, quarter-sliced table) streams 128-edge
    tiles; attention p = exp(lrelu(s_src + d_dst)) with d expanded via
    one-hot matmul; segment-sum via one-hot matmul into per-128-dst-block
    PSUM; denominators via baked ones-columns; biases pre-folded.
  - layer-2 dense (W2cat) fused into layer-1 block epilogue; second
    AllGather; same machinery with H=1; classifier + log_softmax fused.
"""
import sys
sys.path.insert(0, "/opt/trn_rl_repo")
import numpy as np

import concourse.bass as bass
import concourse.bacc as bacc
import concourse.tile as tile
from concourse import mybir, library_config
from concourse import bass_utils

P = 128
NEG_SLOPE = 0.2
N_NODES = 100000
FIN = 165
HID = 64
H1 = 4
NC_OUT = 2
NCORES = 8
NPC = N_NODES // NCORES          # 12500
NBLK = (NPC + P - 1) // P        # 98
QROWS = 25000
NQ = 4
CH = NPC // NQ                   # 3125: local rows per table chunk
RU1 = 384                        # fp16 units per L1 row (768B)
RU2 = 128                        # fp16 units per L2 row (256B)
S1F = 130                        # f32 col of s in L1 row (unit 260)
S2F = 33                         # f32 col of s2 in L2 row (unit 66)
GM = 8                           # tiles per e/p/M batch

f16 = mybir.dt.float16
f32 = mybir.dt.float32
f8 = mybir.dt.float8e4
i16 = mybir.dt.int16
AF = mybir.ActivationFunctionType
ALU = mybir.AluOpType
AXX = mybir.AxisListType.X


def _host_prep(edge_index):
    src = np.concatenate([np.asarray(edge_index[0]), np.arange(N_NODES)]).astype(np.int64)
    dst = np.concatenate([np.asarray(edge_index[1]), np.arange(N_NODES)]).astype(np.int64)
    core_of = dst // NPC
    percore = []
    cnt = np.zeros((NCORES, NBLK, NQ), np.int64)
    for k in range(NCORES):
        sel = np.nonzero(core_of == k)[0]
        s_k = src[sel]
        d_k = dst[sel] - k * NPC
        blk = d_k >> 7
        # chunk-major table: node n sits at chunk c=(n%NPC)//CH, local idx
        # (n//NPC)*CH + (n%NPC)%CH — lets AllGather chunk c start as soon as
        # local rows [c*CH,(c+1)*CH) exist
        q = (s_k % NPC) // CH
        # within (block, chunk): ascending table address for DRAM locality
        perm_idx = (s_k // NPC) * CH + (s_k % NPC) % CH
        order = np.lexsort((perm_idx, q, blk))
        s_k, d_k, blk, q = s_k[order], d_k[order], blk[order], q[order]
        percore.append((s_k, d_k, blk, q))
        np.add.at(cnt[k], (blk, q), 1)
    reg = cnt.max(axis=0)                       # shared padded count per (b,q)
    tiles_bq = (reg + P - 1) // P               # may be 0
    ntiles = int(tiles_bq.sum())
    calls = [(b, qq, int(tiles_bq[b, qq]))
             for b in range(NBLK) for qq in range(NQ) if tiles_bq[b, qq] > 0]
    plan = {"reg": reg, "tiles_bq": tiles_bq, "ntiles": ntiles,
            "ncalls": len(calls)}

    data = []
    for k in range(NCORES):
        s_k, d_k, blk, q = percore[k]
        srcs = np.zeros(ntiles * P, np.int64)
        dloc = np.full(ntiles * P, -1, np.int64)
        gcnt = np.zeros(len(calls), np.int32)
        t0 = 0
        ci = 0
        # per-(b,q): real edges then a -1 (skipped) pad tail
        ptr = {}
        for b in range(NBLK):
            for qq in range(NQ):
                T = int(tiles_bq[b, qq])
                if T == 0:
                    continue
                m = (blk == b) & (q == qq)
                cs = s_k[m]; cd = d_k[m]
                L = len(cs)
                base = t0 * P
                # chunk-local permuted index (0..QROWS), -1 for skipped pads
                srcs[base:base + L] = (cs // NPC) * CH + (cs % NPC) % CH
                dloc[base:base + L] = cd & 127
                srcs[base + L:base + T * P] = -1
                gcnt[ci] = L
                ci += 1
                ptr[(b, qq)] = (t0, T)
                t0 += T
        assert t0 == ntiles and ci == len(calls)
        srcs_t = srcs.reshape(ntiles, P)
        dloc_t = dloc.reshape(ntiles, P)

        # D^T [edge j, row i] and D [row i, edge j] per tile, packed [P, 256];
        # stored p-major [P, ntiles, 256] so per-block DMA reads are one long
        # contiguous span per partition (no sub-512B descriptor penalty)
        import ml_dtypes
        DD = np.zeros((ntiles, P, 2 * P), ml_dtypes.float8_e4m3)
        for t in range(ntiles):
            dl = dloc_t[t]
            v = np.nonzero(dl >= 0)[0]
            DD[t, v, dl[v]] = 1.0            # cols 0:128  = D^T
            DD[t, dl[v], P + v] = 1.0        # cols 128:256 = D
        DD = np.ascontiguousarray(DD.transpose(1, 0, 2))
        # int16 quarter-local idx, wrapped [16, n/16] then replicated to 128
        idx16 = np.zeros((16, ntiles * P // 16), np.int16)
        for b in range(NBLK):
            for qq in range(NQ):
                if (b, qq) not in ptr:
                    continue
                t0_, T = ptr[(b, qq)]
                n = T * P
                flat = srcs_t.reshape(-1)[t0_ * P: t0_ * P + n]
                assert (flat >= -1).all() and (flat < QROWS).all()
                idx16[:, t0_ * P // 16:(t0_ * P + n) // 16] = \
                    flat.reshape(n // 16, 16).T.astype(np.int16)
        data.append(dict(idx_stream=np.tile(idx16, (8, 1)), DD=DD,
                         gcnt=gcnt.reshape(1, -1)))
    return plan, data


def _host_weights(W1, a_src1, a_dst1, W2, a_src2, a_dst2):
    # layer-1 h features stored head-minor: row unit c*H1+h for c<64,
    # ones at 256:260, s-proj at 260+h, d-proj at 264+h
    W1 = np.asarray(W1, np.float32); W2 = np.asarray(W2, np.float32)
    a_src1 = np.asarray(a_src1, np.float32); a_dst1 = np.asarray(a_dst1, np.float32)
    a_src2 = np.asarray(a_src2, np.float32); a_dst2 = np.asarray(a_dst2, np.float32)
    Wcat1 = np.zeros((FIN, 268), np.float32)
    for h in range(H1):
        Wcat1[:, h:256:H1] = W1[:, h * HID:(h + 1) * HID]
        Wcat1[:, 260 + h] = W1[:, h * HID:(h + 1) * HID] @ a_src1[h]
        Wcat1[:, 264 + h] = W1[:, h * HID:(h + 1) * HID] @ a_dst1[h]
    Wcat2 = np.zeros((H1 * HID, 66), np.float32)
    # W2 rows permuted to the (c,h) feature order
    perm = np.arange(H1 * HID).reshape(H1, HID).T.reshape(-1)  # [c*H1+h] -> h*HID+c
    Wcat2[:, :HID] = W2[perm]
    Wcat2[:, 64] = W2[perm] @ a_src2[0]
    Wcat2[:, 65] = W2[perm] @ a_dst2[0]
    return Wcat1.astype(np.float16), Wcat2.astype(np.float16)


def _build(plan, sim=False):
    reg = plan["reg"]; tiles_bq = plan["tiles_bq"]; ntiles = plan["ntiles"]

    nc = bacc.Bacc("TRN2", target_bir_lowering=False, debug=False,
                   enable_asserts=False, num_devices=1 if sim else NCORES,
                   num_swdge_queues=4)

    x_in = nc.dram_tensor("xT", [FIN, NPC], f16, kind="ExternalInput")
    w1_in = nc.dram_tensor("w1cat", [FIN, 268], f16, kind="ExternalInput")
    w2_in = nc.dram_tensor("w2cat", [H1 * HID, 66], f16, kind="ExternalInput")
    b1_in = nc.dram_tensor("b1", [1, H1 * HID], f32, kind="ExternalInput")
    b2_in = nc.dram_tensor("b2", [1, HID], f32, kind="ExternalInput")
    wc_in = nc.dram_tensor("wc", [1, HID * NC_OUT], f32, kind="ExternalInput")
    bc_in = nc.dram_tensor("bc", [1, NC_OUT], f32, kind="ExternalInput")
    idx_in = nc.dram_tensor("idxs", [P, ntiles * P // 16], i16, kind="ExternalInput")
    dd_in = nc.dram_tensor("dds", [P, ntiles, 2 * P], f8, kind="ExternalInput")
    gcnt_in = nc.dram_tensor("gcnt", [1, plan["ncalls"]], mybir.dt.int32,
                             kind="ExternalInput")
    out_t = nc.dram_tensor("out", [NPC, NC_OUT], f32, kind="ExternalOutput")

    RG = [list(range(NCORES))]

    with tile.TileContext(nc) as tc:
        with tc.tile_pool(name="const", bufs=1) as cp, \
             tc.tile_pool(name="work", bufs=3) as wp, \
             tc.tile_pool(name="gst", bufs=3) as gp, \
             tc.tile_pool(name="dts", bufs=8) as dp, \
             tc.tile_pool(name="dram", bufs=1, space="DRAM") as dr, \
             tc.tile_pool(name="psA", bufs=3, space="PSUM") as psA, \
             tc.tile_pool(name="psB", bufs=2, space="PSUM") as psB, \
             tc.tile_pool(name="psC", bufs=2, space="PSUM") as psC, \
             tc.tile_pool(name="psD", bufs=1, space="PSUM") as psD:

            nc.gpsimd.load_library(library_config.mlp)

            hs1_loc = dr.tile([NPC, RU1], f16)
            hs1_full = dr.tile([N_NODES, RU1], f16)
            hs2_loc = dr.tile([NPC, RU2], f16)
            hs2_full = dr.tile([N_NODES, RU2], f16)

            # ---------- constants
            from concourse.masks import make_identity
            ident = cp.tile([P, P], f32)
            make_identity(nc, ident[:])
            w1c = cp.tile([P, 268], f16)
            w1c2 = cp.tile([P, 268], f16)
            nc.sync.dma_start(out=w1c[:], in_=w1_in[0:128, :])
            nc.sync.dma_start(out=w1c2[0:FIN - 128, :], in_=w1_in[128:FIN, :])
            w2c = cp.tile([P, 66], f16)
            w2c2 = cp.tile([P, 66], f16)
            nc.sync.dma_start(out=w2c[:], in_=w2_in[0:128, :])
            nc.sync.dma_start(out=w2c2[:], in_=w2_in[128:256, :])
            onecol = cp.tile([1, P], f16)
            nc.vector.memset(onecol[:], 1.0)
            gcnt_t = cp.tile([1, plan["ncalls"]], mybir.dt.int32)
            nc.sync.dma_start(out=gcnt_t[:], in_=gcnt_in[:])
            gregs = [nc.alloc_register(mybir.EngineType.Pool, f"gcnt_r{i}")
                     for i in range(4)]

            def replicate(dram_ap, ncols, tag):
                srcf = wp.tile([1, 256], f16, tag="repf16")
                srci = wp.tile([1, 256], f32, tag="repf32")
                nc.sync.dma_start(out=srci[:, :ncols], in_=dram_ap)
                nc.vector.tensor_copy(out=srcf[:, :ncols], in_=srci[:, :ncols])
                ps = psD.tile([P, 512], f32, tag="scr")
                nc.tensor.matmul(out=ps[:, :ncols], lhsT=onecol[:], rhs=srcf[:, :ncols],
                                 start=True, stop=True)
                dst = cp.tile([P, ncols], f32, tag=tag)
                nc.vector.tensor_copy(out=dst[:], in_=ps[:, :ncols])
                return dst

            b1rep = replicate(b1_in[:], H1 * HID, "b1rep")
            b2rep = replicate(b2_in[:], HID, "b2rep")
            wcrep = replicate(wc_in[:], HID * NC_OUT, "wcrep")
            bcrep = replicate(bc_in[:], NC_OUT, "bcrep")

            logits = cp.tile([P, NBLK * NC_OUT], f32)

            # ================= pass 0 =================
            for nt in range(NBLK):
                r0 = nt * P
                rows = min(P, NPC - r0)
                xT1 = wp.tile([P, P], f16, tag="xT1")
                xT2 = wp.tile([P, P], f16, tag="xT2")
                if rows < P:
                    nc.vector.memset(xT1[:], 0.0)
                    nc.vector.memset(xT2[:], 0.0)
                nc.sync.dma_start(out=xT1[:, 0:rows], in_=x_in[0:P, r0:r0 + rows])
                nc.sync.dma_start(out=xT2[0:FIN - P, 0:rows],
                                  in_=x_in[P:FIN, r0:r0 + rows])
                acc = psA.tile([P, 268], f32, tag="agg1")
                nc.tensor.matmul(out=acc[:], lhsT=xT1[:], rhs=w1c[:], start=True, stop=False)
                nc.tensor.matmul(out=acc[:], lhsT=xT2[0:FIN - P, :], rhs=w1c2[0:FIN - P, :],
                                 start=False, stop=True)
                row = wp.tile([P, RU1], f16, tag="row1")
                nc.vector.tensor_tensor(
                    out=row[:, 0:256], in0=acc[:, 0:256], in1=b1rep[:], op=ALU.add)
                if nt < 3:
                    # row tile cycles 3 pool buffers; constants persist after
                    nc.vector.memset(row[:, 256:260], 1.0)
                    nc.vector.memset(row[:, 272:RU1], 0.0)
                rowf = row[:].bitcast(f32)
                nc.vector.tensor_copy(out=rowf[:, S1F:S1F + H1], in_=acc[:, 260:264])
                nc.vector.tensor_copy(out=row[:, 268:268 + H1], in_=acc[:, 264:268])
                nc.sync.dma_start(out=hs1_loc[r0:r0 + rows, :], in_=row[0:rows, :])

            # NOTE: strided (row-prefix) collectives fail in walrus neff_packager
            for c in range(NQ):
                if sim:
                    nc.sync.dma_start(
                        out=hs1_full[c * QROWS:c * QROWS + CH, :],
                        in_=hs1_loc[c * CH:(c + 1) * CH, :])
                else:
                    nc.gpsimd.collective_compute(
                        "AllGather", ALU.bypass, replica_groups=RG,
                        ins=[hs1_loc[c * CH:(c + 1) * CH, :]],
                        outs=[hs1_full[c * QROWS:(c + 1) * QROWS, :]])

            # ================= shared epilogue helpers =================
            def _elu(dst, src_ap, ncols, tagp):
                mn = wp.tile([P, ncols], f32, tag=f"{tagp}mn")
                nc.vector.tensor_scalar(mn[:], src_ap, 0.0, None, ALU.min)
                ex = wp.tile([P, ncols], f32, tag=f"{tagp}ex")
                nc.scalar.activation(out=ex[:], in_=mn[:], func=AF.Exp)
                nc.vector.tensor_scalar(ex[:], ex[:], 1.0, None, ALU.subtract)
                nc.vector.tensor_tensor(out=dst, in0=src_ap, in1=ex[:], op=ALU.max)

            def _epilogue1(b, r0, rows, aggp):
                rec = wp.tile([P, H1], f32, tag="rec1")
                nc.vector.reciprocal(out=rec[:], in_=aggp[:, 256:260])
                nrm = wp.tile([P, H1 * HID], f32, tag="nrm1")
                nc.vector.tensor_tensor(
                    out=nrm[:].rearrange("p (c h) -> p c h", h=H1),
                    in0=aggp[:, 0:256].rearrange("p (c h) -> p c h", h=H1),
                    in1=rec[:].unsqueeze(1).to_broadcast([P, HID, H1]),
                    op=ALU.mult)
                h2in = wp.tile([P, H1 * HID], f32, tag="h2in")
                _elu(h2in[:], nrm[:], H1 * HID, "e1")
                pt1 = psD.tile([P, 512], f32, tag="scr")
                h2T1 = wp.tile([P, P], f16, tag="h2T1")
                nc.tensor.transpose(out=pt1[:, 0:P], in_=h2in[:, 0:P], identity=ident[:])
                nc.vector.tensor_copy(out=h2T1[:], in_=pt1[:, 0:P])
                pt2 = psD.tile([P, 512], f32, tag="scr")
                h2T2 = wp.tile([P, P], f16, tag="h2T2")
                nc.tensor.transpose(out=pt2[:, 0:P], in_=h2in[:, P:2 * P], identity=ident[:])
                nc.vector.tensor_copy(out=h2T2[:], in_=pt2[:, 0:P])
                mm2 = psB.tile([P, 66], f32, tag="agg2")
                nc.tensor.matmul(out=mm2[:], lhsT=h2T1[:], rhs=w2c[:], start=True, stop=False)
                nc.tensor.matmul(out=mm2[:], lhsT=h2T2[:], rhs=w2c2[:], start=False, stop=True)
                row2 = wp.tile([P, RU2], f16, tag="row2")
                nc.vector.tensor_tensor(out=row2[:, 0:HID], in0=mm2[:, 0:HID],
                                        in1=b2rep[:], op=ALU.add)
                if b < 3:
                    nc.vector.memset(row2[:, HID:HID + 2], 1.0)
                    nc.vector.memset(row2[:, 69:RU2], 0.0)
                row2f = row2[:].bitcast(f32)
                nc.vector.tensor_copy(out=row2f[:, S2F:S2F + 1], in_=mm2[:, 64:65])
                nc.vector.tensor_copy(out=row2[:, 68:69], in_=mm2[:, 65:66])
                nc.sync.dma_start(out=hs2_loc[r0:r0 + rows, :], in_=row2[0:rows, :])

            def _epilogue2(b, r0, rows, aggp):
                rec = wp.tile([P, 1], f32, tag="rec2")
                nc.vector.reciprocal(out=rec[:], in_=aggp[:, 64:65])
                nrm = wp.tile([P, HID], f32, tag="nrm2")
                nc.vector.tensor_tensor(
                    out=nrm[:], in0=aggp[:, 0:HID],
                    in1=rec[:].to_broadcast([P, HID]), op=ALU.mult)
                h3 = wp.tile([P, HID], f32, tag="h3")
                _elu(h3[:], nrm[:], HID, "e2")
                tmp = wp.tile([P, HID], f32, tag="lgt")
                wcv = wcrep[:].rearrange("p (k c) -> p k c", c=NC_OUT)
                lg = logits[:, b * NC_OUT:(b + 1) * NC_OUT]
                for c in range(NC_OUT):
                    nc.vector.tensor_tensor(out=tmp[:], in0=h3[:],
                                            in1=wcv[:, :, c], op=ALU.mult)
                    nc.vector.tensor_reduce(out=lg[:, c:c + 1], in_=tmp[:],
                                            op=ALU.add, axis=AXX)
                nc.vector.tensor_tensor(out=lg, in0=lg, in1=bcrep[:], op=ALU.add)

            def _final_logsoftmax():
                # batched log_softmax over logits [P, NBLK, NC_OUT]
                lgv = logits[:].rearrange("p (b c) -> p b c", c=NC_OUT)
                mx = wp.tile([P, NBLK], f32, tag="fmx")
                nc.vector.tensor_reduce(out=mx[:].unsqueeze(2), in_=lgv,
                                        op=ALU.max, axis=AXX)
                nc.vector.tensor_tensor(
                    out=lgv, in0=lgv,
                    in1=mx[:].unsqueeze(2).to_broadcast([P, NBLK, NC_OUT]),
                    op=ALU.subtract)
                exs = wp.tile([P, NBLK * NC_OUT], f32, tag="fexs")
                nc.scalar.activation(out=exs[:], in_=logits[:], func=AF.Exp)
                sm = wp.tile([P, NBLK], f32, tag="fsm")
                nc.vector.tensor_reduce(
                    out=sm[:].unsqueeze(2),
                    in_=exs[:].rearrange("p (b c) -> p b c", c=NC_OUT),
                    op=ALU.add, axis=AXX)
                lsm = wp.tile([P, NBLK], f32, tag="flsm")
                nc.scalar.activation(out=lsm[:], in_=sm[:], func=AF.Ln)
                nc.vector.tensor_tensor(
                    out=lgv, in0=lgv,
                    in1=lsm[:].unsqueeze(2).to_broadcast([P, NBLK, NC_OUT]),
                    op=ALU.subtract)

            # ================= edge loops =================
            maxTb = int(tiles_bq.sum(axis=1).max())

            def edge_layer(layer):
                RU = RU1 if layer == 1 else RU2
                NH = H1 if layer == 1 else 1
                MW = 260 if layer == 1 else 65
                SF = S1F if layer == 1 else S2F
                loc_tab = hs1_loc if layer == 1 else hs2_loc
                DOFF = 268 if layer == 1 else 68
                table = hs1_full if layer == 1 else hs2_full
                Tb_list = [int(tiles_bq[bb].sum()) for bb in range(NBLK)]
                t_glob = 0
                ci = 0
                for b in range(NBLK):
                    Tb = Tb_list[b]
                    G = gp.tile([P, maxTb, RU], f16, tag=f"G{layer}")
                    if b < 3:
                        # init all cycled buffers so skipped pad slots hold
                        # finite values (later blocks inherit real rows)
                        nc.vector.memset(G[:], 0.0)
                    idxs = dp.tile([P, (maxTb * P) // 16], i16, tag=f"ix{layer}")
                    nc.sync.dma_start(
                        out=idxs[:, 0:(Tb * P) // 16],
                        in_=idx_in[:, (t_glob * P) // 16:
                                   ((t_glob + Tb) * P) // 16])
                    ddb = gp.tile([P, maxTb, 2 * P], f8, tag=f"DDB{layer}")
                    nc.sync.dma_start(
                        out=ddb[:, 0:Tb, :],
                        in_=dd_in[:, t_glob:t_glob + Tb, :])
                    poff = 0
                    tt = 0
                    for qq in range(NQ):
                        T = int(tiles_bq[b, qq])
                        if T == 0:
                            continue
                        cv = gregs[ci % 4]
                        nc.gpsimd.reg_load(cv, gcnt_t[0:1, ci:ci + 1])
                        ci += 1
                        nc.gpsimd.dma_gather(
                            G[:, tt:tt + T, :],
                            table[qq * QROWS:(qq + 1) * QROWS, :],
                            idxs[:, ((poff + tt) * P) // 16:
                                 ((poff + tt + T) * P) // 16],
                            T * P, cv, RU,
                            queue_num=(b * NQ + qq) % 4,
                        )
                        tt += T
                    dblk = wp.tile([P, NH], f16, tag=f"dblk{layer}")
                    drows = min(P, NPC - b * P)
                    if drows < P:
                        nc.vector.memset(dblk[:], 0.0)
                    nc.sync.dma_start(
                        out=dblk[0:drows, :],
                        in_=loc_tab[b * P:b * P + drows, DOFF:DOFF + NH])
                    aggp = (psA if layer == 1 else psB).tile(
                        [P, 268 if layer == 1 else 66], f32,
                        tag="agg1" if layer == 1 else "agg2")

                    t = 0
                    while t < Tb:
                        gmt = min(GM, Tb - t)
                        dxp = psC.tile([P, GM * H1], f32, tag="dx")
                        ddg = ddb[:, poff:poff + Tb, :]
                        for ti in range(gmt):
                            nc.tensor.matmul(
                                out=dxp[:, ti * NH:(ti + 1) * NH],
                                lhsT=ddg[:, t + ti, P:2 * P], rhs=dblk[:],
                                start=True, stop=True, skip_group_check=True)
                        ee = wp.tile([P, GM * NH], f32, tag=f"e{layer}")
                        Gf = G[:].bitcast(f32)
                        nc.vector.tensor_tensor(
                            out=ee[:, :gmt * NH],
                            in0=Gf[:, t:t + gmt, SF:SF + NH],
                            in1=dxp[:, :gmt * NH], op=ALU.add)
                        lr = wp.tile([P, GM * NH], f32, tag=f"lr{layer}")
                        nc.scalar.activation(out=lr[:, :gmt * NH], in_=ee[:, :gmt * NH],
                                             func=AF.Prelu, alpha=NEG_SLOPE)
                        # f16 so the M multiply hits DVE 2x 16-bit mode
                        pp = wp.tile([P, GM * NH], f16, tag=f"pp{layer}")
                        nc.scalar.activation(out=pp[:, :gmt * NH], in_=lr[:, :gmt * NH],
                                             func=AF.Exp)
                        M = wp.tile([P, GM, MW], f16, tag=f"M{layer}")
                        if layer == 1:
                            # head-minor (c,h) layout: broadcast on middle dim
                            # keeps last dim packed -> DVE 2x/4x 16-bit mode
                            nc.vector.tensor_tensor(
                                out=M[:, 0:gmt, :].rearrange(
                                    "p t (c h) -> p t c h", h=NH),
                                in0=G[:, t:t + gmt, 0:MW].rearrange(
                                    "p t (c h) -> p t c h", h=NH),
                                in1=pp[:].rearrange("p (t h) -> p t h", h=NH)
                                    [:, 0:gmt, :].unsqueeze(2)
                                    .to_broadcast([P, gmt, 65, NH]),
                                op=ALU.mult)
                        else:
                            nc.vector.tensor_tensor(
                                out=M[:, 0:gmt, :],
                                in0=G[:, t:t + gmt, 0:MW],
                                in1=pp[:, 0:gmt].unsqueeze(2)
                                    .to_broadcast([P, gmt, MW]),
                                op=ALU.mult)
                        for ti in range(gmt):
                            nc.tensor.matmul(
                                out=aggp[:, 0:MW], lhsT=ddg[:, t + ti, 0:P],
                                rhs=M[:, ti, :],
                                start=(t + ti == 0), stop=(t + ti == Tb - 1),
                                skip_group_check=True)
                        t += gmt

                    r0 = b * P
                    rows = min(P, NPC - r0)
                    if layer == 1:
                        _epilogue1(b, r0, rows, aggp)
                    else:
                        _epilogue2(b, r0, rows, aggp)
                    t_glob += Tb

            edge_layer(1)
            for c in range(NQ):
                if sim:
                    nc.sync.dma_start(
                        out=hs2_full[c * QROWS:c * QROWS + CH, :],
                        in_=hs2_loc[c * CH:(c + 1) * CH, :])
                else:
                    nc.gpsimd.collective_compute(
                        "AllGather", ALU.bypass, replica_groups=RG,
                        ins=[hs2_loc[c * CH:(c + 1) * CH, :]],
                        outs=[hs2_full[c * QROWS:(c + 1) * QROWS, :]])
            edge_layer(2)
            _final_logsoftmax()

            nc.sync.dma_start(
                out=bass.AP(out_t[:].tensor, 0,
                            [[NC_OUT, P], [P * NC_OUT, NBLK - 1], [1, NC_OUT]]),
                in_=logits[:, 0:(NBLK - 1) * NC_OUT])
            lastrows = NPC - (NBLK - 1) * P
            nc.sync.dma_start(
                out=bass.AP(out_t[:].tensor, (NBLK - 1) * P * NC_OUT,
                            [[NC_OUT, lastrows], [1, NC_OUT]]),
                in_=logits[0:lastrows, (NBLK - 1) * NC_OUT:NBLK * NC_OUT])

    nc.compile()
    return nc


_CACHE = {}


def kernel(**inputs):
    x = np.asarray(inputs["x"], np.float32)
    edge_index = np.asarray(inputs["edge_index"])
    ekey = (edge_index.shape, int(edge_index[:, ::997].astype(np.int64).sum()))
    if _CACHE.get("ekey") != ekey:
        plan, data = _host_prep(edge_index)
        _CACHE.update(ekey=ekey, plan=plan, data=data, nc=None)
    plan, data = _CACHE["plan"], _CACHE["data"]
    Wcat1, Wcat2 = _host_weights(
        inputs["W1"], inputs["a_src1"], inputs["a_dst1"],
        inputs["W2"], inputs["a_src2"], inputs["a_dst2"])
    perm = np.arange(H1 * HID).reshape(H1, HID).T.reshape(-1)
    b1 = np.asarray(inputs["b1"], np.float32).reshape(-1)[perm].reshape(1, -1)
    b2 = np.asarray(inputs["b2"], np.float32).reshape(1, -1)
    wc = np.asarray(inputs["Wc"], np.float32).reshape(1, -1).copy()
    bc = np.asarray(inputs["bc"], np.float32).reshape(1, -1)

    if _CACHE.get("nc") is None:
        _CACHE["nc"] = _build(plan)
    nc = _CACHE["nc"]
    in_maps = []
    for k in range(NCORES):
        in_maps.append({
            "xT": np.ascontiguousarray(
                x[k * NPC:(k + 1) * NPC].T).astype(np.float16),
            "w1cat": Wcat1, "w2cat": Wcat2,
            "b1": b1, "b2": b2, "wc": wc, "bc": bc,
            "idxs": data[k]["idx_stream"],
            "dds": data[k]["DD"],
            "gcnt": data[k]["gcnt"],
        })
    res = bass_utils.run_bass_kernel_spmd(
        nc, in_maps, core_ids=list(range(NCORES)),
        trace=globals().get("TRACE", False))
    globals()["LAST_RES"] = res
    globals()["LAST_NC"] = nc
    globals()["LAST_IN_MAPS"] = in_maps
    out = np.concatenate([np.asarray(r["out"], np.float32) for r in res.results], axis=0)
    return out


if __name__ == "__main__":
    # smoke test with tiny synthetic graph shape is not supported (shapes
    # hardcoded); run via test.py on the real problem.
    pass

